# revision 7
# baseline (speedup 1.0000x reference)
"""AWing loss kernel for Trainium2 (8 NeuronCores, pure data parallel).

Problem (hardcoded): prediction/target f32 [32, 68, 128, 128] -> scalar f32
    loss = mean(awing(pred, tgt) * mask),  mask = 1 + 10*[dilate3x3(tgt) > 0.2]

Branch-free math (exact):
    d   = |p - t|
    dc  = clamp(d, 0, 0.5)
    e   = 2.1 - t
    EZ  = dc^e = exp(e*ln(dc))          # = d^e (d<.5) or 0.5^e (d>=.5)
    SP  = ln(1+EZ)                      # softplus branch-merge
    E2  = exp(-SP) = 1/(1+EZ)
    q2R = (1-E2)*(4.2-2t)*relu(d-0.5) = 2*(1-E2)*(2.1-t)*relu(d-0.5)
    m in {1,11}
    result = 14/N * (sum(m*SP) + 2*sum((E2-1)*(t-2.1)*m*relu(d-0.5)))

Engine assignment (HW-measured: Pool elementwise is ~29us/op -> banned;
ACT ~2.0us, DVE f32 ~2.2us / bf16-TT ~1.1us / bf16-TS ~0.6us per
[128,2048] op; DMA ~400 GB/s on contiguous 16KB/partition descriptors):
  ACT (one table set, natural_log_exp_and_others; no table switches):
      tb=Copy(t), L=Ln(d), ez=Exp(-zn), sp=Ln(1+ez), e2=Exp(-sp),
      sg=Sign(cs-0.5)
  DVE (bf16): x=p-t, d=abs_max(x,0), rdmr=max(d,.5)-.5, ind=[tb>0.2],
      u=tb-2.1, zn=min(L,-ln2)*u, mt=5*sg+6, rm=mt*rdmr, g2=u*rm,
      dump1=(e2-1)*g2 (+acc), dump2=mt*sp (+acc)
  PE: 3x3 dilation count = tri(h) x 3 shifted accumulating matmuls over
      zero-padded-in-w indicator (zero pad == SAME-truncated window).

This toolchain's walrus encodes at most ONE sync wait per instruction;
Tile emits more. _fission_multiwaits() splits surplus waits onto NoOps
inserted before the offending instruction on the same engine.

Sharding: batch dim 32 -> 4 batches (272 (b,c) planes) per core.
Host pre-transposes to [H, PPC, 2, W] so every SBUF partition (h) reads
one contiguous 16 KB chunk per tile DMA.
"""

import numpy as np
from contextlib import ExitStack

B, C, H, W = 32, 68, 128, 128
NCORES = 8
PPC = (B // NCORES) * C          # 272 planes per core
NP = 16                          # planes per SBUF tile
NT = PPC // NP                   # 17 tiles per core
F = NP * W                       # 2048 free elements per partition per tile
N_TOTAL = B * C * H * W
LN2 = 0.6931471805599453

_CACHE = {}


def _build_nc(repeat=1, loop_reps=0):
    import concourse.bass as bass
    import concourse.mybir as mybir
    import ml_dtypes
    from concourse.tile import TileContext

    f32 = mybir.dt.float32
    bf16 = mybir.dt.bfloat16
    Alu = mybir.AluOpType
    Act = mybir.ActivationFunctionType

    nc = bass.Bass(num_swdge_queues=1)
    # Host pre-transposes to [H, PPC, 2, W]: every SBUF partition (h) then
    # reads one contiguous 16 KB chunk per tile (128 big descriptors at
    # line rate) instead of 32 strided 512 B chunks (descriptor-bound).
    pt_d = nc.dram_tensor("pt", [H, PPC, 2, W], f32, kind="ExternalInput")
    out_d = nc.dram_tensor("out", [128, 1], f32, kind="ExternalOutput")

    # Tridiagonal-ones [128,128]: (tri @ x)[h] = x[h-1]+x[h]+x[h+1] (SAME).
    tri_np = np.zeros((H, H), dtype=ml_dtypes.bfloat16)
    for i in range(H):
        for j2 in range(max(0, i - 1), min(H, i + 2)):
            tri_np[i, j2] = 1.0
    tri_d = nc.inline_tensor(tri_np, name="tri")

    # const APs for ACT biases (pre-created; same pattern Bass uses
    # internally, but outside the TileContext)
    for dt_, vals in ((f32, (0.0, -0.5)), (bf16, (0.0, 1.0))):
        for v in vals:
            nm = f"const-{'f32' if dt_ is f32 else 'bf16'}-{v}"
            _c = nc.alloc_sbuf_tensor(nm, [128, 1], dt_)
            nc.gpsimd.memset(_c.ap(), v)
            nc.const_aps.aps[(dt_, v)] = _c.ap()
    nc.all_engine_barrier()

    with TileContext(nc) as tc, ExitStack() as ctx:
        cpool = ctx.enter_context(tc.tile_pool(name="cpool", bufs=1))
        io = ctx.enter_context(tc.tile_pool(name="io", bufs=2))
        wk = ctx.enter_context(tc.tile_pool(name="wk", bufs=2))
        psp = ctx.enter_context(tc.tile_pool(name="psp", bufs=2, space="PSUM"))

        tri_s = cpool.tile([H, H], bf16, name="tri_s")
        nc.sync.dma_start(tri_s[:], tri_d[:, :])
        acc1 = cpool.tile([128, NT], f32, name="acc1")
        acc2 = cpool.tile([128, NT], f32, name="acc2")

        # zero the w-pad columns of both ind_pad buffers once (zero pad ==
        # "false" indicator == SAME-truncated dilation window)
        for _ in range(2):
            ip = wk.tile([128, NP, W + 2], bf16, name="indp", tag="indp")
            nc.vector.memset(ip[:, :, 0:1], 0.0)
            nc.vector.memset(ip[:, :, W + 1:W + 2], 0.0)

        import contextlib
        loop_cm = tc.For_i(0, loop_reps, 1) if loop_reps else contextlib.nullcontext()
        with loop_cm:
            for j in [jj for _ in range(repeat) for jj in range(NT)]:
                # one DMA per tile: [128(h), NP, 2(p/t), W], straight slice of
                # the host-transposed layout -> 16 KB contiguous per partition
                pts = io.tile([128, NP, 2, W], f32, name="pts", tag="pts")
                nc.sync.dma_start(
                    pts[:], pt_d[:, j * NP:(j + 1) * NP, :, :])
                ptv = pts[:, :, 0, :]
                ttv = pts[:, :, 1, :]

                # tb = bf16(t)  (ACT; Copy keeps float-imm bias)
                tb = wk.tile([128, NP, W], bf16, name="tb", tag="tb", bufs=1)
                nc.scalar.activation(tb[:], ttv, Act.Copy)
                # x = p - t  (DVE f32-in, bf16 out)
                x = wk.tile([128, NP, W], bf16, name="x", tag="x", bufs=1)
                nc.vector.tensor_tensor(x[:], ptv, ttv, Alu.subtract)
                # d = |x| = max(-x, x)   (DVE bf16 STT; abs_max ALU is
                # rejected by this execution path)
                dab = wk.tile([128, NP, W], bf16, name="dab", tag="dab")
                nc.vector.scalar_tensor_tensor(
                    dab[:], x[:], -1.0, x[:], Alu.mult, Alu.max)
                # rdmr = relu(d-0.5) = max(d,0.5) - 0.5   (DVE bf16 TS)
                rdmr = wk.tile([128, NP, W], bf16, name="rdmr", tag="rdmr")
                nc.vector.tensor_scalar(rdmr[:], dab[:], 0.5, -0.5,
                                        Alu.max, Alu.add)
                # ind = [t > 0.2] into the padded indicator tile (DVE TS)
                ind_pad = wk.tile([128, NP, W + 2], bf16, name="indp",
                                  tag="indp")
                nc.vector.tensor_scalar(ind_pad[:, :, 1:W + 1], tb[:], 0.2,
                                        None, Alu.is_gt)
                # u = t - 2.1   (DVE bf16 TS)
                u = wk.tile([128, NP, W], bf16, name="u", tag="u")
                nc.vector.tensor_scalar(u[:], tb[:], 2.1, None, Alu.subtract)

                # L = ln(d)   (ACT; ln(0) -> -inf propagates correctly)
                L = wk.tile([128, NP, W], bf16, name="L", tag="L", bufs=1)
                nc.scalar.activation(L[:], dab[:], Act.Ln)
                # zn = min(L, -ln2) * u = -e*ln(dc) >= 0   (DVE bf16 STT)
                zn = wk.tile([128, NP, W], bf16, name="zn", tag="zn")
                nc.vector.scalar_tensor_tensor(
                    zn[:], L[:], -LN2, u[:], Alu.min, Alu.mult)
                # ez = exp(-zn) = dc^e
                ez = wk.tile([128, NP, W], bf16, name="ez", tag="ez", bufs=1)
                nc.scalar.activation(ez[:], zn[:], Act.Exp, scale=-1.0)
                # sp = ln(1 + ez)
                sp = wk.tile([128, NP, W], bf16, name="sp", tag="sp")
                nc.scalar.activation(sp[:], ez[:], Act.Ln, bias=1.0)
                # e2 = exp(-sp) = 1/(1+ez)
                e2 = wk.tile([128, NP, W], bf16, name="e2", tag="e2")
                nc.scalar.activation(e2[:], sp[:], Act.Exp, scale=-1.0)

                # 3x3 dilation count: tri(h-dir) x 3 shifted accumulating
                # matmuls (w-dir) over the zero-padded indicator -> PSUM
                cs = psp.tile([128, F], f32, name="cs", tag="cs")
                for c in range(F // 512):
                    for k in range(3):
                        nc.tensor.matmul(
                            cs[:, c * 512:(c + 1) * 512], tri_s[:],
                            ind_pad[:, c * 4:(c + 1) * 4, k:k + W],
                            start=(k == 0), stop=(k == 2))
                # sg = sign(cs-0.5) in {-1,1}   (ACT reads PSUM)
                sg = wk.tile([128, NP, W], bf16, name="sg", tag="sg", bufs=1)
                csv = cs[:].rearrange("h (a b) -> h a b", a=NP)
                nc.scalar.activation(sg[:], csv, Act.Sign, bias=-0.5)
                # mt = 5*sg+6 in {1,11}   (DVE bf16 TS)
                mt = wk.tile([128, NP, W], bf16, name="mt", tag="mt")
                nc.vector.tensor_scalar(mt[:], sg[:], 5.0, 6.0,
                                        Alu.mult, Alu.add)

                # rm = m * relu(d-1/2)          (DVE bf16 TT)
                rm = wk.tile([128, NP, W], bf16, name="rm", tag="rm", bufs=1)
                nc.vector.tensor_tensor(rm[:], mt[:], rdmr[:], Alu.mult)
                # g2 = (t-2.1) * rm             (DVE bf16 TT)
                g2 = wk.tile([128, NP, W], bf16, name="g2", tag="g2", bufs=1)
                nc.vector.tensor_tensor(g2[:], u[:], rm[:], Alu.mult)
                # acc2[:, j] = sum((e2-1) * g2) = sum(m*(1-E2)*(2.1-t)*relu)
                dump1 = wk.tile([128, NP, W], bf16, name="dump1", tag="dumpb",
                                bufs=1)
                nc.vector.scalar_tensor_tensor(
                    dump1[:], e2[:], 1.0, g2[:], Alu.subtract, Alu.mult,
                    accum_out=acc2[:, j:j + 1])
                # acc1[:, j] = sum(m * SP)
                dump2 = wk.tile([128, NP, W], bf16, name="dump2", tag="dumpb",
                                bufs=1)
                nc.vector.scalar_tensor_tensor(
                    dump2[:], mt[:], 1.0, sp[:], Alu.mult, Alu.mult,
                    accum_out=acc1[:, j:j + 1])

        # result = sum(acc1) + 2*sum(acc2)
        tot = cpool.tile([128, NT], f32, name="tot")
        nc.vector.scalar_tensor_tensor(
            tot[:], acc2[:], 2.0, acc1[:], Alu.mult, Alu.add)
        vec = cpool.tile([128, 1], f32, name="vec")
        nc.vector.tensor_reduce(
            vec[:], tot[:], axis=mybir.AxisListType.X, op=Alu.add)
        nc.sync.dma_start(out_d[:, :], vec[:])

    _fission_multiwaits(nc, mybir)
    return nc


def _fission_multiwaits(nc, mybir):
    """walrus here encodes at most ONE sync wait per instruction; Tile emits
    more. Split: surplus waits move to NoOps inserted just before the
    instruction on the same engine (program order preserves semantics)."""
    nid = [0]

    def mk_nop(engine, wait):
        nid[0] += 1
        nop = mybir.InstNoOp(name=f"WF-{nid[0]}", ins=[], outs=[])
        nop.engine = engine
        nop.sync_info = mybir.SyncInfo(on_wait=[wait], on_update=[])
        return nop

    for f in nc.m.functions:
        for bb in f.blocks:
            out = []
            for ins in bb.instructions:
                si = getattr(ins, "sync_info", None)
                if si is not None and len(si.on_wait) > 1:
                    waits = list(si.on_wait)
                    for w in waits[:-1]:
                        out.append(mk_nop(ins.engine, w))
                    ins.sync_info = mybir.SyncInfo(
                        on_wait=[waits[-1]], on_update=list(si.on_update))
                out.append(ins)
            bb.instructions[:] = out


def _get_nc():
    if "nc" not in _CACHE:
        _CACHE["nc"] = _build_nc()
    return _CACHE["nc"]


def prep_inmaps(prediction, target):
    p = np.asarray(prediction, dtype=np.float32).reshape(NCORES, PPC, H, W)
    t = np.asarray(target, dtype=np.float32).reshape(NCORES, PPC, H, W)
    stacked = np.stack([p, t], axis=2)  # [NCORES, PPC, 2, H, W]
    # host-side transpose to [NCORES, H, PPC, 2, W] so the device DMA is a
    # plain affine slice with 16 KB contiguous per partition (see _build_nc)
    arr = np.ascontiguousarray(stacked.transpose(0, 3, 1, 2, 4))
    return [{"pt": arr[c]} for c in range(NCORES)]


def run(prediction, target, trace=False, **trace_kw):
    from concourse.bass_utils import run_bass_kernel_spmd

    nc = _get_nc()
    in_maps = prep_inmaps(prediction, target)
    res = run_bass_kernel_spmd(
        nc, in_maps, core_ids=list(range(NCORES)), trace=trace, **trace_kw)
    total = 0.0
    for r in res.results:
        total += np.asarray(r["out"], dtype=np.float64).sum()
    value = np.float32(14.0 * total / N_TOTAL)
    return value, res


def kernel(prediction, target):
    value, _ = run(prediction, target)
    return value


# revision 14
# speedup vs baseline: 5.8065x; 5.8065x over previous
"""AWing loss kernel for Trainium2 (8 NeuronCores, pure data parallel).

Problem (hardcoded): prediction/target f32 [32, 68, 128, 128] -> scalar f32
    loss = mean(awing(pred, tgt) * mask),  mask = 1 + 10*[dilate3x3(tgt) > 0.2]

Branch-free math (exact):
    d   = |p - t|
    dc  = clamp(d, 0, 0.5)
    e   = 2.1 - t
    EZ  = dc^e = exp(e*ln(dc))          # = d^e (d<.5) or 0.5^e (d>=.5)
    SP  = ln(1+EZ)                      # softplus branch-merge
    E2  = exp(-SP) = 1/(1+EZ)
    q2R = (1-E2)*(4.2-2t)*relu(d-0.5) = 2*(1-E2)*(2.1-t)*relu(d-0.5)
    m in {1,11}
    result = 14/N * (sum(m*SP) + 2*sum((E2-1)*(t-2.1)*m*relu(d-0.5)))

Engine assignment (HW-measured per [128,2048] op: Pool elementwise
~29us -> banned; ACT ~2.0us; DVE TT-f32/STT 1x ~2.3us, TT-bf16 2x
~1.2us, TS-bf16 4x ~0.8us, TS-f32 2x ~1.2us; matmul[128x128@128x512]
~0.4us; DMA ~400 GB/s contiguous. STT has NO 2x mode -> avoided):
  ACT (one table set, natural_log_exp_and_others; no table switches):
      d=Abs(x), L=Ln(d), ez=Exp(-zn), sp=Ln(1+ez), e2=Exp(-sp),
      sg=Sign(cs-0.5)
  DVE (bf16 TS/TT only): x=p-t, rdmr=max(d,.5)-.5, u=t-2.1 (f32 2x),
      ind=[u>-1.9], Lcm=min(L,-ln2), zn=Lcm*u, mt=10*sg+12 (=2m),
      rm=mt*rdmr, g2=u*rm, e2g2=e2*g2, mtsp=mt*sp
  PE: 3x3 dilation count = tri(h) x 3 shifted accumulating matmuls over
      zero-padded-in-w indicator (zero pad == SAME-truncated window);
      ALL loss reductions as matmuls with constant lhsT into one PSUM
      bank accumulated across the whole pass:
      total = sum(0.5*mtsp) + sum(e2g2) - sum(g2)
            = sum(m*SP) + 2*sum((E2-1)*(t-2.1)*m*relu(d-.5)).

This toolchain's walrus encodes at most ONE sync wait per instruction;
Tile emits more. _fission_multiwaits() splits surplus waits onto NoOps
inserted before the offending instruction on the same engine.

Sharding: batch dim 32 -> 4 batches (272 (b,c) planes) per core.
Host pre-transposes to [H, PPC, 2, W] so every SBUF partition (h) reads
one contiguous 16 KB chunk per tile DMA.
"""

import numpy as np
from contextlib import ExitStack

B, C, H, W = 32, 68, 128, 128
NCORES = 8
PPC = (B // NCORES) * C          # 272 planes per core
NP = 16                          # planes per SBUF tile
NT = PPC // NP                   # 17 tiles per core
F = NP * W                       # 2048 free elements per partition per tile
N_TOTAL = B * C * H * W
LN2 = 0.6931471805599453

_CACHE = {}


def _build_nc(repeat=1, loop_reps=0):
    import concourse.bass as bass
    import concourse.mybir as mybir
    import ml_dtypes
    from concourse.tile import TileContext

    f32 = mybir.dt.float32
    bf16 = mybir.dt.bfloat16
    Alu = mybir.AluOpType
    Act = mybir.ActivationFunctionType

    nc = bass.Bass(num_swdge_queues=1)
    # Host pre-transposes to [H, PPC, 2, W]: every SBUF partition (h) then
    # reads one contiguous 16 KB chunk per tile (128 big descriptors at
    # line rate) instead of 32 strided 512 B chunks (descriptor-bound).
    pt_d = nc.dram_tensor("pt", [H, PPC, 2, W], f32, kind="ExternalInput")
    out_d = nc.dram_tensor("out", [128, 1], f32, kind="ExternalOutput")

    # Tridiagonal-ones [128,128]: (tri @ x)[h] = x[h-1]+x[h]+x[h+1] (SAME).
    tri_np = np.zeros((H, H), dtype=ml_dtypes.bfloat16)
    for i in range(H):
        for j2 in range(max(0, i - 1), min(H, i + 2)):
            tri_np[i, j2] = 1.0
    tri_d = nc.inline_tensor(tri_np, name="tri")
    # Constant lhsT matrices for PE loss reductions: out[m,f] = w*sum_h rhs
    half_d = nc.inline_tensor(
        np.full((H, H), 0.5, dtype=ml_dtypes.bfloat16), name="chalf")
    ones_d = nc.inline_tensor(
        np.full((H, H), 1.0, dtype=ml_dtypes.bfloat16), name="cones")
    mones_d = nc.inline_tensor(
        np.full((H, H), -1.0, dtype=ml_dtypes.bfloat16), name="cmones")

    # const APs for ACT biases (pre-created; same pattern Bass uses
    # internally, but outside the TileContext)
    for dt_, vals in ((f32, (0.0, -0.5)), (bf16, (0.0, 1.0))):
        for v in vals:
            nm = f"const-{'f32' if dt_ is f32 else 'bf16'}-{v}"
            _c = nc.alloc_sbuf_tensor(nm, [128, 1], dt_)
            nc.gpsimd.memset(_c.ap(), v)
            nc.const_aps.aps[(dt_, v)] = _c.ap()
    nc.all_engine_barrier()

    with TileContext(nc) as tc, ExitStack() as ctx:
        cpool = ctx.enter_context(tc.tile_pool(name="cpool", bufs=1))
        io = ctx.enter_context(tc.tile_pool(name="io", bufs=2))
        wk = ctx.enter_context(tc.tile_pool(name="wk", bufs=2))
        # PSUM: mask counts 4 banks (bufs=1) + 1 bank for the loss reduction
        psp = ctx.enter_context(tc.tile_pool(name="psp", bufs=1, space="PSUM"))
        psr = ctx.enter_context(tc.tile_pool(name="psr", bufs=1, space="PSUM"))

        tri_s = cpool.tile([H, H], bf16, name="tri_s")
        nc.sync.dma_start(tri_s[:], tri_d[:, :])
        half_s = cpool.tile([H, H], bf16, name="half_s")
        nc.sync.dma_start(half_s[:], half_d[:, :])
        ones_s = cpool.tile([H, H], bf16, name="ones_s")
        nc.sync.dma_start(ones_s[:], ones_d[:, :])
        mones_s = cpool.tile([H, H], bf16, name="mones_s")
        nc.sync.dma_start(mones_s[:], mones_d[:, :])

        # zero the w-pad columns of both ind_pad buffers once (zero pad ==
        # "false" indicator == SAME-truncated dilation window)
        for _ in range(2):
            ip = wk.tile([128, NP, W + 2], bf16, name="indp", tag="indp")
            nc.vector.memset(ip[:, :, 0:1], 0.0)
            nc.vector.memset(ip[:, :, W + 1:W + 2], 0.0)

        # one PSUM bank accumulates every loss term across the whole pass
        rsum = psr.tile([128, 512], f32, name="rsum")

        import contextlib
        loop_cm = tc.For_i(0, loop_reps, 1) if loop_reps else contextlib.nullcontext()
        tiles = [jj for _ in range(repeat) for jj in range(NT)]
        with loop_cm:
            for jn, j in enumerate(tiles):
                first = jn == 0
                last = jn == len(tiles) - 1
                # one DMA per tile: [128(h), NP, 2(p/t), W], straight slice of
                # the host-transposed layout -> 16 KB contiguous per partition
                pts = io.tile([128, NP, 2, W], f32, name="pts", tag="pts")
                nc.sync.dma_start(
                    pts[:], pt_d[:, j * NP:(j + 1) * NP, :, :])
                ptv = pts[:, :, 0, :]
                ttv = pts[:, :, 1, :]

                # x = p - t  (DVE f32-in TT 1x, bf16 out)
                x = wk.tile([128, NP, W], bf16, name="x", tag="x", bufs=1)
                nc.vector.tensor_tensor(x[:], ptv, ttv, Alu.subtract)
                # u = t - 2.1  (DVE f32-in TS 2x, bf16 out)
                u = wk.tile([128, NP, W], bf16, name="u", tag="u")
                nc.vector.tensor_scalar(u[:], ttv, 2.1, None, Alu.subtract)
                # ind = [t > 0.2] == [u > -1.9] into the padded indicator
                # tile (DVE bf16 TS 4x)
                ind_pad = wk.tile([128, NP, W + 2], bf16, name="indp",
                                  tag="indp")
                nc.vector.tensor_scalar(ind_pad[:, :, 1:W + 1], u[:], -1.9,
                                        None, Alu.is_gt)
                # d = |x|  (ACT)
                dab = wk.tile([128, NP, W], bf16, name="dab", tag="dab")
                nc.scalar.activation(dab[:], x[:], Act.Abs)
                # rdmr = relu(d-0.5) = max(d,0.5) - 0.5   (DVE bf16 TS 4x)
                rdmr = wk.tile([128, NP, W], bf16, name="rdmr", tag="rdmr")
                nc.vector.tensor_scalar(rdmr[:], dab[:], 0.5, -0.5,
                                        Alu.max, Alu.add)

                # L = ln(d)   (ACT; ln(0) -> -inf propagates correctly)
                L = wk.tile([128, NP, W], bf16, name="L", tag="L", bufs=1)
                nc.scalar.activation(L[:], dab[:], Act.Ln)
                # Lcm = min(L, -ln2) = ln(dc)   (DVE bf16 TS 4x)
                Lcm = wk.tile([128, NP, W], bf16, name="Lcm", tag="Lcm",
                              bufs=1)
                nc.vector.tensor_scalar(Lcm[:], L[:], -LN2, None, Alu.min)
                # zn = Lcm * u = -e*ln(dc) >= 0   (DVE bf16 TT 2x)
                zn = wk.tile([128, NP, W], bf16, name="zn", tag="zn")
                nc.vector.tensor_tensor(zn[:], Lcm[:], u[:], Alu.mult)
                # ez = exp(-zn) = dc^e
                ez = wk.tile([128, NP, W], bf16, name="ez", tag="ez", bufs=1)
                nc.scalar.activation(ez[:], zn[:], Act.Exp, scale=-1.0)
                # sp = ln(1 + ez)
                sp = wk.tile([128, NP, W], bf16, name="sp", tag="sp")
                nc.scalar.activation(sp[:], ez[:], Act.Ln, bias=1.0)
                # e2 = exp(-sp) = 1/(1+ez)
                e2 = wk.tile([128, NP, W], bf16, name="e2", tag="e2")
                nc.scalar.activation(e2[:], sp[:], Act.Exp, scale=-1.0)

                # 3x3 dilation count: tri(h-dir) x 3 shifted accumulating
                # matmuls (w-dir) over the zero-padded indicator -> PSUM
                cs = psp.tile([128, F], f32, name="cs", tag="cs")
                for c in range(F // 512):
                    for k in range(3):
                        nc.tensor.matmul(
                            cs[:, c * 512:(c + 1) * 512], tri_s[:],
                            ind_pad[:, c * 4:(c + 1) * 4, k:k + W],
                            start=(k == 0), stop=(k == 2))
                # sg = sign(cs-0.5) in {-1,1}   (ACT reads PSUM)
                sg = wk.tile([128, NP, W], bf16, name="sg", tag="sg", bufs=1)
                csv = cs[:].rearrange("h (a b) -> h a b", a=NP)
                nc.scalar.activation(sg[:], csv, Act.Sign, bias=-0.5)
                # mt = 10*sg+12 = 2m in {2,22}   (DVE bf16 TS 4x)
                mt = wk.tile([128, NP, W], bf16, name="mt", tag="mt")
                nc.vector.tensor_scalar(mt[:], sg[:], 10.0, 12.0,
                                        Alu.mult, Alu.add)

                # rm = 2m * relu(d-1/2)          (DVE bf16 TT 2x)
                rm = wk.tile([128, NP, W], bf16, name="rm", tag="rm", bufs=1)
                nc.vector.tensor_tensor(rm[:], mt[:], rdmr[:], Alu.mult)
                # g2 = (t-2.1) * rm              (DVE bf16 TT 2x)
                g2 = wk.tile([128, NP, W], bf16, name="g2", tag="g2")
                nc.vector.tensor_tensor(g2[:], u[:], rm[:], Alu.mult)
                # e2g2 = e2 * g2                 (DVE bf16 TT 2x)
                e2g2 = wk.tile([128, NP, W], bf16, name="e2g2", tag="e2g2")
                nc.vector.tensor_tensor(e2g2[:], e2[:], g2[:], Alu.mult)
                # mtsp = 2m * sp                 (DVE bf16 TT 2x)
                mtsp = wk.tile([128, NP, W], bf16, name="mtsp", tag="mtsp")
                nc.vector.tensor_tensor(mtsp[:], mt[:], sp[:], Alu.mult)

                # loss reductions on PE: rsum += 0.5*col_sum(mtsp)
                # + col_sum(e2g2) - col_sum(g2), chunks folded into the
                # same 512 columns; one accumulation group per pass
                prods = [(half_s, mtsp), (ones_s, e2g2), (mones_s, g2)]
                for pi, (lhs, prod) in enumerate(prods):
                    pv = prod[:].rearrange("h a b -> h (a b)")
                    for c in range(F // 512):
                        nc.tensor.matmul(
                            rsum[:, :], lhs[:],
                            pv[:, c * 512:(c + 1) * 512],
                            start=(first and pi == 0 and c == 0),
                            stop=(last and pi == 2 and c == 3),
                            skip_group_check=True)

        # every partition of rsum holds identical per-column partial sums
        vec = cpool.tile([128, 1], f32, name="vec")
        nc.vector.tensor_reduce(
            vec[:], rsum[:], axis=mybir.AxisListType.X, op=Alu.add)
        nc.sync.dma_start(out_d[:, :], vec[:])

    _fission_multiwaits(nc, mybir)
    return nc


def _fission_multiwaits(nc, mybir):
    """walrus here encodes at most ONE sync wait per instruction; Tile emits
    more. Split: surplus waits move to NoOps inserted just before the
    instruction on the same engine (program order preserves semantics)."""
    nid = [0]

    def mk_nop(engine, wait):
        nid[0] += 1
        nop = mybir.InstNoOp(name=f"WF-{nid[0]}", ins=[], outs=[])
        nop.engine = engine
        nop.sync_info = mybir.SyncInfo(on_wait=[wait], on_update=[])
        return nop

    for f in nc.m.functions:
        for bb in f.blocks:
            out = []
            for ins in bb.instructions:
                si = getattr(ins, "sync_info", None)
                if si is not None and len(si.on_wait) > 1:
                    waits = list(si.on_wait)
                    for w in waits[:-1]:
                        out.append(mk_nop(ins.engine, w))
                    ins.sync_info = mybir.SyncInfo(
                        on_wait=[waits[-1]], on_update=list(si.on_update))
                out.append(ins)
            bb.instructions[:] = out


def _get_nc():
    if "nc" not in _CACHE:
        _CACHE["nc"] = _build_nc()
    return _CACHE["nc"]


def prep_inmaps(prediction, target):
    p = np.asarray(prediction, dtype=np.float32).reshape(NCORES, PPC, H, W)
    t = np.asarray(target, dtype=np.float32).reshape(NCORES, PPC, H, W)
    stacked = np.stack([p, t], axis=2)  # [NCORES, PPC, 2, H, W]
    # host-side transpose to [NCORES, H, PPC, 2, W] so the device DMA is a
    # plain affine slice with 16 KB contiguous per partition (see _build_nc)
    arr = np.ascontiguousarray(stacked.transpose(0, 3, 1, 2, 4))
    return [{"pt": arr[c]} for c in range(NCORES)]


def run(prediction, target, trace=False, **trace_kw):
    from concourse.bass_utils import run_bass_kernel_spmd

    nc = _get_nc()
    in_maps = prep_inmaps(prediction, target)
    res = run_bass_kernel_spmd(
        nc, in_maps, core_ids=list(range(NCORES)), trace=trace, **trace_kw)
    total = 0.0
    for r in res.results:
        total += np.asarray(r["out"], dtype=np.float64).sum()
    # every partition row repeats the per-core total -> divide by 128
    value = np.float32(14.0 * total / (N_TOTAL * 128.0))
    return value, res


def kernel(prediction, target):
    value, _ = run(prediction, target)
    return value
